# revision 2
# baseline (speedup 1.0000x reference)
"""Trainium2 Bass kernel for nn_Attention_54580444397738 (gnn_message_passing).

Math per batch b (B=8, N=128, H=256, C=16):
  proj         = local @ W_apair                                     [N, H]
  pre[i,j,:]   = proj[i,:] + proj[j,:] + binary[i,j,:] @ W_binary
                 + b_apair + b_binary                                [N, N, H]
  score[i,j]   = sigmoid(relu(pre[i,j,:]) . W_att + b_att)           [N, N]
  glob         = score @ local                                       [N, H]
  local_pair [i,j,:] = local[i,:] + local[j,:]                       (output 1)
  global_pair[i,j,:] = glob[i,:]  + glob[j,:]                        (output 2)

Sharding: data-parallel over batch B across the 8 cores (1 batch per core).
Memory-bound: 32 MiB of output stores per core at ~330 GB/s measured ->
~100 us floor; everything else is engineered to hide behind the store
stream.

Design (vs the j-major compensated-exact baseline, 404 us -> 288 us):
  - i-major output orientation: each output chunk is [i=128 partitions,
    (j h)] so every partition's store slice is contiguous in DRAM (4 KB
    descriptors, 0.5 MB stores, 64 per output tensor, 8-deep stage
    rotation).  Store-pattern probe measured 314-338 GB/s/core for all
    layouts; what matters is keeping the queue full, which the per-chunk
    store granularity + deep stage pool does.
  - Output chunk [128, 4*H] = X[i,:] (exact f32, DVE free-dim broadcast)
    + X[j,:] rows broadcast to all partitions with a single ones-row f32r
    matmul into PSUM (~1e-4 rel error, tolerance 2e-2 -> the baseline's
    compensation pair is dropped).  Adds are split DVE (D-chunks) / ACT
    via full-PSUM E-chunks to spread engine load.
  - Attention keeps the baseline's validated PSUM accumulation order
    (row-mm 512 / per-half identR + binT) -- any reorder crashes the
    device -- but binT/wx4/relu/affine_mul_reduce run in bf16, halving
    the DVE reduce cost.  Attention interleaves with phase-1 chunks on a
    16:7 cadence that completes ~8 chunks before the phase-1 store tail
    so glob (sigmoid + score @ local) resolves without stalling phase 2.
  - Flat row staging (rows flattened per 32-row group at partitions
    {0,32,64,96} for the ones-row matmuls): flatX loads straight from the
    DRAM input in one strided SWDGE cast DMA; projFlat/flatG bounce
    through DRAM (SBUF->SBUF partition folding is silently wrong) and
    reload with one strided DMA each.
  - Known HW quirks honored: f32r consumers need producers that declare
    f32r output (bitcasts are rejected by the BIR verifier); matmul
    operands need 32-aligned base partitions; GPSIMD cannot touch PSUM;
    partition_broadcast reads partition 0 only.
"""

import numpy as np

B, N, H, BIN = 8, 128, 256, 16
NCORES = 8
CPAD = 32        # c dim padded 16 -> 32 so transposed blocks land 32-aligned
IG = 4           # i's per binary-transpose group (4 * 32 = 128)
JBLK = 16        # j's per stage (2 MB stages)
CHUNK = 4        # j's per generated chunk (2 PSUM banks)
PROLOG = 8       # attention steps deferred until this many gen chunks ran
STAGGER_BINP = False

VAR_P1 = ("D", "E", "D", "D")   # phase-1 chunk variants (PE busy with attn)
VAR_P2 = ("E", "D", "E", "D")   # phase-2 chunk variants

_cache = {}


def _body(tc, io, reps=1):
    import concourse.bass as bass
    import concourse.mybir as mybir
    from concourse.masks import make_identity
    from contextlib import ExitStack, nullcontext

    nc = tc.nc
    ts = bass.ts
    f32 = mybir.dt.float32
    f32r = mybir.dt.float32r
    bf16 = mybir.dt.bfloat16
    Relu = mybir.ActivationFunctionType.Relu
    Sigmoid = mybir.ActivationFunctionType.Sigmoid

    local_d, binary_d, wap_d, bap_d, wbin_d, bbin_d, watt_d, batt_d, lp_d, gp_d = io

    lp_flat = lp_d.rearrange("i j h -> i (j h)")
    gp_flat = gp_d.rearrange("i j h -> i (j h)")

    ctx = ExitStack()
    with ctx:
        persist = ctx.enter_context(tc.tile_pool(name="persist", bufs=1))
        binTp = ctx.enter_context(tc.tile_pool(name="binTp", bufs=6))
        att2p = ctx.enter_context(tc.tile_pool(name="att2p", bufs=6))
        stagep = ctx.enter_context(tc.tile_pool(name="stagep", bufs=8))
        prep = ctx.enter_context(tc.tile_pool(name="prep", bufs=3, space="PSUM"))
        genp = ctx.enter_context(tc.tile_pool(name="genp", bufs=2, space="PSUM"))
        outpp = ctx.enter_context(tc.tile_pool(name="outpp", bufs=1, space="PSUM"))
        dramp = ctx.enter_context(tc.tile_pool(name="dramp", bufs=1, space="DRAM"))

        # timing builds wrap the whole body in a device-side loop
        loop = tc.For_i(0, reps, 1) if reps > 1 else nullcontext()
        ctx.enter_context(loop)

        # ---------------- persistent setup ----------------
        identity = persist.tile([128, 128], f32, tag="identity")
        make_identity(nc, identity)
        identR = persist.tile([128, 128], f32r, tag="identR")
        nc.vector.tensor_copy(out=identR, in_=identity)
        onesF = persist.tile([128, 128], f32, tag="onesF")
        nc.gpsimd.memset(onesF, 1.0)
        onesT = persist.tile([128, 128], f32r, tag="onesT")
        nc.vector.tensor_copy(out=onesT, in_=onesF)

        localSb = persist.tile([N, H], f32, tag="localSb")
        nc.sync.dma_start(out=localSb, in_=local_d)

        # flatX[32q, r*H:(r+1)*H] = X[32q + r]  (one strided cast-load, f32r)
        flatX = persist.tile([97, 32 * H], f32r, tag="flatX")
        nc.gpsimd.dma_start(
            out=flatX[0:97:32, :], in_=local_d.rearrange("(a x) h -> a (x h)", a=4)
        )

        # f32r weights (cast during SWDGE load)
        wap0 = persist.tile([128, H], f32r, tag="wap0")
        nc.gpsimd.dma_start(out=wap0, in_=wap_d[0:128])
        wap1 = persist.tile([128, H], f32r, tag="wap1")
        nc.gpsimd.dma_start(out=wap1, in_=wap_d[128:256])

        biasA = persist.tile([1, H], f32, tag="biasA")
        nc.sync.dma_start(out=biasA, in_=bap_d.unsqueeze(0))
        biasB = persist.tile([1, H], f32, tag="biasB")
        nc.sync.dma_start(out=biasB, in_=bbin_d.unsqueeze(0))
        biasRow = persist.tile([1, H], bf16, tag="biasRow")
        nc.vector.tensor_add(out=biasRow, in0=biasA, in1=biasB)

        wbinSb = persist.tile([16, H], f32, tag="wbinSb")
        nc.sync.dma_start(out=wbinSb, in_=wbin_d)
        wbinB = persist.tile([16, H], bf16, tag="wbinB")
        nc.vector.tensor_copy(out=wbinB, in_=wbinSb)

        wattRow = persist.tile([1, H], f32, tag="wattRow")
        nc.sync.dma_start(out=wattRow, in_=watt_d.rearrange("k o -> o k"))
        wattRowB = persist.tile([1, H], bf16, tag="wattRowB")
        nc.vector.tensor_copy(out=wattRowB, in_=wattRow)
        battRow = persist.tile([1, 1], f32, tag="battRow")
        nc.sync.dma_start(out=battRow, in_=batt_d.unsqueeze(0))

        # Wx4: W_binary + bias row replicated at partitions {0,32,64,96} (bf16)
        wx4 = persist.tile([128, H], bf16, tag="wx4")
        for m in range(4):
            nc.sync.dma_start(out=wx4[32 * m : 32 * m + 16, :], in_=wbinB)
            nc.sync.dma_start(out=wx4[32 * m + 16 : 32 * m + 17, :], in_=biasRow)

        # broadcast W_att across partitions (bf16); b_att as a [128,1] column
        wattB = persist.tile([128, H], bf16, tag="wattB")
        battCol = persist.tile([128, 1], f32, tag="battCol")
        nc.gpsimd.partition_broadcast(wattB, wattRowB)
        nc.gpsimd.partition_broadcast(battCol, battRow)

        # localT = local^T (f32r), then projW = local @ W_apair (f32r)
        localT = persist.tile([128, H], f32r, tag="localT")
        for hb in range(2):
            tp = outpp.tile([128, H], f32, tag="outp")
            nc.tensor.transpose(tp[:, 0:128], localSb[:, ts(hb, 128)], identity)
            nc.scalar.copy(out=localT[:, ts(hb, 128)], in_=tp[:, 0:128])
        pp = outpp.tile([128, H], f32, tag="outp")
        nc.tensor.matmul(pp, lhsT=localT[:, 0:128], rhs=wap0, start=True, stop=False)
        nc.tensor.matmul(pp, lhsT=localT[:, 128:256], rhs=wap1, start=False, stop=True)
        projW = persist.tile([128, H], f32r, tag="projW")
        nc.scalar.copy(out=projW, in_=pp)
        projWr = projW

        # projFlat: bounce projW through DRAM, reload folded (one strided DMA)
        projDram = dramp.tile([N, H], f32r, tag="projDram")
        nc.sync.dma_start(out=projDram, in_=projW)
        projFlat = persist.tile([97, 32 * H], f32r, tag="projFlat")
        nc.sync.dma_start(
            out=projFlat[0:97:32, :],
            in_=projDram.rearrange("(a x) h -> a (x h)", a=4),
        )

        # binp[q][j, (i32, c32)]: c 0..15 = binary[., i, j, .], c16 = 1.0 (bias
        # lane).  Four separate quarter tiles so their loads can stagger into
        # the chunk loop without false dependencies.
        binp = []
        for q in range(4):
            bq = persist.tile([128, 32 * CPAD], f32, tag=f"binp{q}")
            nc.gpsimd.memset(bq, 0.0)
            nc.gpsimd.memset(
                bq.rearrange("p (i c) -> p i c", c=CPAD)[:, :, 16:17], 1.0
            )
            binp.append(bq)

        def load_binp(q):
            nc.sync.dma_start(
                out=binp[q].rearrange("p (i c) -> p i c", c=CPAD)[:, :, 0:BIN],
                in_=binary_d[ts(q, 32)].rearrange("i j c -> j i c"),
            )

        logits = persist.tile([128, N], f32, tag="logits")
        ttrS = persist.tile([128, H], bf16, tag="ttrS")
        binT = {}

        # ---------------- helpers ----------------
        def row_rhs(flat, i, width):
            q, r = divmod(i, 32)
            return flat[32 * q : 32 * q + 1, r * H : r * H + width]

        def row_lhsT(i):
            q = i // 32
            return onesT[32 * q : 32 * q + 1, :]

        def row_tp(i):
            return (32 * (i // 32), 0)

        projW2 = projWr.unsqueeze(1).broadcast_to([128, 2, H])

        def attn_step(i):
            g, il = divmod(i, IG)
            if il == 0:  # transpose this binary group: [j,(i4,c32)] -> [(i4,c32),j]
                tp = outpp.tile([128, H], f32, tag="outp")
                nc.tensor.transpose(
                    tp[:, 0:128], binp[g // 8][:, ts(g % 8, 128)], identity
                )
                bt = binTp.tile([128, 128], bf16, tag="binT")
                nc.scalar.copy(out=bt, in_=tp[:, 0:128])
                binT[g] = bt
            if i % 2 == 1:
                return
            pre = prep.tile([128, 2 * H], f32, tag="pre")
            nc.tensor.matmul(pre, lhsT=row_lhsT(i), rhs=row_rhs(projFlat, i, 2 * H),
                             start=True, stop=False, tile_position=row_tp(i))
            for m in range(2):
                ii = i + m
                gg, iil = divmod(ii, IG)
                nc.tensor.matmul(pre[:, ts(m, H)], lhsT=identR, rhs=projWr,
                                 start=False, stop=False)
                nc.tensor.matmul(
                    pre[:, ts(m, H)],
                    lhsT=binT[gg][32 * iil : 32 * iil + 17, :],
                    rhs=wx4[32 * iil : 32 * iil + 17, :],
                    start=False, stop=(m == 1), tile_position=(32 * iil, 0),
                )
            a2 = att2p.tile([128, 2 * H], bf16, tag="att2")
            nc.scalar.activation(out=a2, in_=pre, func=Relu)
            for m in range(2):
                nc.vector.affine_mul_reduce(
                    out=ttrS, accum_out=logits[:, i + m : i + m + 1],
                    in0=a2[:, ts(m, H)], in1=wattB, scale=1.0, bias=0.0,
                )

        # ---------------- output phase ----------------
        # binp quarter q feeds attention i-ticks [32q, 32q+32); issue its load
        # a few chunks ahead of first use.
        BINP_AT = {8: 1, 22: 2, 36: 3} if STAGGER_BINP else {}

        def out_phase(xSb, xR, src_flat, dram_flat, with_attn):
            x4 = xSb.unsqueeze(1).broadcast_to([128, CHUNK, H])
            x2 = (xR if xR is not None else xSb).unsqueeze(1).broadcast_to(
                [128, 2, H])
            variants = VAR_P1 if with_attn else VAR_P2
            attn_at = 0

            def attn_tick(limit):
                nonlocal attn_at
                if with_attn:
                    while attn_at < min(limit, N):
                        attn_step(attn_at)
                        attn_at += 1

            for nchunk in range(N // CHUNK):
                jj = CHUNK * nchunk
                v = variants[nchunk % len(variants)]
                po = genp.tile([128, CHUNK * H], f32, tag="gen")
                for b in range(CHUNK // 2):
                    if v == "E":
                        nc.tensor.matmul(
                            po[:, ts(b, 2 * H)], lhsT=identR, rhs=x2,
                            start=True, stop=False,
                        )
                    nc.tensor.matmul(
                        po[:, ts(b, 2 * H)], lhsT=row_lhsT(jj),
                        rhs=row_rhs(src_flat, jj + 2 * b, 2 * H),
                        start=(v != "E"), stop=True, tile_position=row_tp(jj),
                    )
                stage = stagep.tile([128, CHUNK * H], f32, tag="stage")
                st3 = stage.rearrange("p (j h) -> p j h", h=H)
                po3 = po.rearrange("p (j h) -> p j h", h=H)
                if v == "D":
                    nc.vector.tensor_add(out=st3, in0=x4, in1=po3)
                else:  # E: both terms already in PSUM
                    nc.scalar.copy(out=st3, in_=po3)
                nc.sync.dma_start(
                    out=dram_flat[:, jj * H : (jj + CHUNK) * H], in_=stage
                )
                if with_attn and nchunk in BINP_AT:
                    load_binp(BINP_AT[nchunk])
                # 128 attn i-ticks paced over the first ~56 of 64 chunks so
                # glob resolves before the phase-1 store tail.
                attn_tick(((nchunk + 1) * 5) // 2 - 10)
            attn_tick(N)

        # ---------------- phase 1: local_pair + attention ----------------
        load_binp(0)
        if not STAGGER_BINP:
            for q in (1, 2, 3):
                load_binp(q)
        xR1 = persist.tile([N, H], f32r, tag="xR1")
        nc.vector.tensor_copy(out=xR1, in_=localSb)
        out_phase(localSb, xR1, flatX, lp_flat, with_attn=True)

        # ---------------- scores -> glob ----------------
        scoreT = persist.tile([128, N], f32, tag="scoreT")
        globSb = persist.tile([128, H], f32, tag="globSb")
        nc.scalar.activation(out=scoreT, in_=logits, func=Sigmoid, bias=battCol)
        pg = outpp.tile([128, H], f32, tag="outp")
        nc.tensor.matmul(pg, lhsT=scoreT, rhs=localSb, start=True, stop=True)
        nc.vector.tensor_copy(out=globSb, in_=pg)
        globR = persist.tile([128, H], f32r, tag="globR")
        nc.scalar.copy(out=globR, in_=pg)
        globDram = dramp.tile([N, H], f32r, tag="globDram")
        nc.sync.dma_start(out=globDram, in_=globR)
        flatG = persist.tile([97, 32 * H], f32r, tag="flatG")
        nc.sync.dma_start(
            out=flatG[0:97:32, :],
            in_=globDram.rearrange("(a x) h -> a (x h)", a=4),
        )

        # ---------------- phase 2: global_pair ----------------
        out_phase(globSb, globR, flatG, gp_flat, with_attn=False)


def _build(reps=1):
    import concourse.bass as bass  # noqa: F401
    from concourse import bacc
    import concourse.mybir as mybir
    import concourse.tile as tile

    f32 = mybir.dt.float32
    nc = bacc.Bacc(
        "TRN2",
        target_bir_lowering=False,
        debug=False,
        enable_asserts=False,
        num_devices=NCORES,
    )
    io = (
        nc.dram_tensor("local", [N, H], f32, kind="ExternalInput").ap(),
        nc.dram_tensor("binary", [N, N, BIN], f32, kind="ExternalInput").ap(),
        nc.dram_tensor("w_apair", [H, H], f32, kind="ExternalInput").ap(),
        nc.dram_tensor("b_apair", [H], f32, kind="ExternalInput").ap(),
        nc.dram_tensor("w_binary", [BIN, H], f32, kind="ExternalInput").ap(),
        nc.dram_tensor("b_binary", [H], f32, kind="ExternalInput").ap(),
        nc.dram_tensor("w_att", [H, 1], f32, kind="ExternalInput").ap(),
        nc.dram_tensor("b_att", [1], f32, kind="ExternalInput").ap(),
        nc.dram_tensor("out_lp", [N, N, H], f32, kind="ExternalOutput").ap(),
        nc.dram_tensor("out_gp", [N, N, H], f32, kind="ExternalOutput").ap(),
    )
    with tile.TileContext(nc) as tc:
        _body(tc, io, reps=reps)
    nc.compile()
    return nc


def _get_nc():
    if "nc" not in _cache:
        _cache["nc"] = _build()
    return _cache["nc"]


def _run(inputs, trace=False):
    from concourse.bass_utils import run_bass_kernel_spmd

    nc = _get_nc()
    f = lambda x: np.ascontiguousarray(np.asarray(x), dtype=np.float32)
    shared = {
        "w_apair": f(inputs["W_apair"]),
        "b_apair": f(inputs["b_apair"]),
        "w_binary": f(inputs["W_binary"]),
        "b_binary": f(inputs["b_binary"]),
        "w_att": f(inputs["W_att"]),
        "b_att": f(inputs["b_att"]),
    }
    local = f(inputs["local_feats"])
    binary = f(inputs["binary_feats"])
    in_maps = [
        {"local": local[c], "binary": binary[c], **shared} for c in range(NCORES)
    ]
    res = run_bass_kernel_spmd(
        nc, in_maps, core_ids=list(range(NCORES)), trace=trace
    )
    lp = np.stack([r["out_lp"] for r in res.results])
    gp = np.stack([r["out_gp"] for r in res.results])
    return (lp, gp), res


def kernel(**inputs):
    out, _ = _run(inputs, trace=False)
    return out


# revision 3
# speedup vs baseline: 1.0037x; 1.0037x over previous
"""Trainium2 Bass kernel for nn_Attention_54580444397738 (gnn_message_passing).

Math per batch b (B=8, N=128, H=256, C=16):
  proj         = local @ W_apair                                     [N, H]
  pre[i,j,:]   = proj[i,:] + proj[j,:] + binary[i,j,:] @ W_binary
                 + b_apair + b_binary                                [N, N, H]
  score[i,j]   = sigmoid(relu(pre[i,j,:]) . W_att + b_att)           [N, N]
  glob         = score @ local                                       [N, H]
  local_pair [i,j,:] = local[i,:] + local[j,:]                       (output 1)
  global_pair[i,j,:] = glob[i,:]  + glob[j,:]                        (output 2)

Sharding: data-parallel over batch B across the 8 cores (1 batch per core).
Memory-bound: 32 MiB of output stores per core at ~330 GB/s measured ->
~100 us floor; everything else is engineered to hide behind the store
stream.

Design (vs the j-major compensated-exact baseline, 404 us -> 288 us):
  - i-major output orientation: each output chunk is [i=128 partitions,
    (j h)] so every partition's store slice is contiguous in DRAM (4 KB
    descriptors, 0.5 MB stores, 64 per output tensor, 6-deep stage
    rotation).  A store-pattern probe measured 314-338 GB/s/core across
    layouts; per-chunk store granularity + the deep stage pool keep the
    DMA queue full.
  - Output chunk [128, 4*H] = X[i,:] (exact f32, DVE free-dim broadcast)
    + X[j,:] rows broadcast to all partitions with a single ones-row f32r
    matmul into PSUM (~1e-4 rel error, tolerance 2e-2, so the baseline's
    exactness-compensation matmul pair is dropped).  Adds split between
    DVE (D-chunks) and ACT via full-PSUM E-chunks to spread engine load.
  - Attention keeps the baseline's validated PSUM accumulation order
    (row-mm 512 / per-half identR + binT) -- any reorder crashes the
    device -- but binT/wx4/relu/affine_mul_reduce run in bf16, halving
    the DVE reduce cost.  Attention interleaves with phase-1 chunks on a
    16:7 cadence that completes several chunks before the phase-1 store
    tail so glob (sigmoid + score @ local) resolves promptly for phase 2.
  - Flat row staging (rows flattened per 32-row group at partitions
    {0,32,64,96} for the ones-row matmuls): flatX loads straight from the
    DRAM input in one strided SWDGE cast DMA; projFlat/flatG bounce
    through DRAM and reload with one strided DMA each (SBUF->SBUF
    partition folding is silently wrong).
  - Known HW quirks honored: f32r consumers need producers that declare
    f32r output (bitcasts are rejected by the BIR verifier); matmul
    operands need 32-aligned base partitions; GPSIMD cannot touch PSUM;
    partition_broadcast reads partition 0 only.
"""

import numpy as np

B, N, H, BIN = 8, 128, 256, 16
NCORES = 8
CPAD = 32        # c dim padded 16 -> 32 so transposed blocks land 32-aligned
IG = 4           # i's per binary-transpose group (4 * 32 = 128)
JBLK = 16        # j's per stage (2 MB stages)
CHUNK = 4        # j's per generated chunk (2 PSUM banks)
PROLOG = 8       # attention steps deferred until this many gen chunks ran
STAGGER_BINP = False

VAR_P1 = ("D", "E", "D", "D")   # phase-1 chunk variants (PE busy with attn)
VAR_P2 = ("E", "D", "E", "D")   # phase-2 chunk variants

_cache = {}


def _body(tc, io, reps=1):
    import concourse.bass as bass
    import concourse.mybir as mybir
    from concourse.masks import make_identity
    from contextlib import ExitStack, nullcontext

    nc = tc.nc
    ts = bass.ts
    f32 = mybir.dt.float32
    f32r = mybir.dt.float32r
    bf16 = mybir.dt.bfloat16
    Relu = mybir.ActivationFunctionType.Relu
    Sigmoid = mybir.ActivationFunctionType.Sigmoid

    local_d, binary_d, wap_d, bap_d, wbin_d, bbin_d, watt_d, batt_d, lp_d, gp_d = io

    lp_flat = lp_d.rearrange("i j h -> i (j h)")
    gp_flat = gp_d.rearrange("i j h -> i (j h)")

    ctx = ExitStack()
    with ctx:
        persist = ctx.enter_context(tc.tile_pool(name="persist", bufs=1))
        binTp = ctx.enter_context(tc.tile_pool(name="binTp", bufs=6))
        att2p = ctx.enter_context(tc.tile_pool(name="att2p", bufs=4))
        stagep = ctx.enter_context(tc.tile_pool(name="stagep", bufs=6))
        prep = ctx.enter_context(tc.tile_pool(name="prep", bufs=2, space="PSUM"))
        genp = ctx.enter_context(tc.tile_pool(name="genp", bufs=2, space="PSUM"))
        outpp = ctx.enter_context(tc.tile_pool(name="outpp", bufs=2, space="PSUM"))
        dramp = ctx.enter_context(tc.tile_pool(name="dramp", bufs=1, space="DRAM"))

        # timing builds wrap the whole body in a device-side loop
        loop = tc.For_i(0, reps, 1) if reps > 1 else nullcontext()
        ctx.enter_context(loop)

        # ---------------- persistent setup ----------------
        identity = persist.tile([128, 128], f32, tag="identity")
        make_identity(nc, identity)
        identR = persist.tile([128, 128], f32r, tag="identR")
        nc.vector.tensor_copy(out=identR, in_=identity)
        onesF = persist.tile([128, 128], f32, tag="onesF")
        nc.gpsimd.memset(onesF, 1.0)
        onesT = persist.tile([128, 128], f32r, tag="onesT")
        nc.vector.tensor_copy(out=onesT, in_=onesF)

        localSb = persist.tile([N, H], f32, tag="localSb")
        nc.sync.dma_start(out=localSb, in_=local_d)

        # flatX[32q, r*H:(r+1)*H] = X[32q + r]  (one strided cast-load, f32r)
        flatX = persist.tile([97, 32 * H], f32r, tag="flatX")
        nc.gpsimd.dma_start(
            out=flatX[0:97:32, :], in_=local_d.rearrange("(a x) h -> a (x h)", a=4)
        )

        # f32r weights (cast during SWDGE load)
        wap0 = persist.tile([128, H], f32r, tag="wap0")
        nc.gpsimd.dma_start(out=wap0, in_=wap_d[0:128])
        wap1 = persist.tile([128, H], f32r, tag="wap1")
        nc.gpsimd.dma_start(out=wap1, in_=wap_d[128:256])

        biasA = persist.tile([1, H], f32, tag="biasA")
        nc.sync.dma_start(out=biasA, in_=bap_d.unsqueeze(0))
        biasB = persist.tile([1, H], f32, tag="biasB")
        nc.sync.dma_start(out=biasB, in_=bbin_d.unsqueeze(0))
        biasRow = persist.tile([1, H], bf16, tag="biasRow")
        nc.vector.tensor_add(out=biasRow, in0=biasA, in1=biasB)

        wbinSb = persist.tile([16, H], f32, tag="wbinSb")
        nc.sync.dma_start(out=wbinSb, in_=wbin_d)
        wbinB = persist.tile([16, H], bf16, tag="wbinB")
        nc.vector.tensor_copy(out=wbinB, in_=wbinSb)

        wattRow = persist.tile([1, H], f32, tag="wattRow")
        nc.sync.dma_start(out=wattRow, in_=watt_d.rearrange("k o -> o k"))
        wattRowB = persist.tile([1, H], bf16, tag="wattRowB")
        nc.vector.tensor_copy(out=wattRowB, in_=wattRow)
        battRow = persist.tile([1, 1], f32, tag="battRow")
        nc.sync.dma_start(out=battRow, in_=batt_d.unsqueeze(0))

        # Wx4: W_binary + bias row replicated at partitions {0,32,64,96} (bf16)
        wx4 = persist.tile([128, H], bf16, tag="wx4")
        for m in range(4):
            nc.sync.dma_start(out=wx4[32 * m : 32 * m + 16, :], in_=wbinB)
            nc.sync.dma_start(out=wx4[32 * m + 16 : 32 * m + 17, :], in_=biasRow)

        # broadcast W_att across partitions (bf16); b_att as a [128,1] column
        wattB = persist.tile([128, H], bf16, tag="wattB")
        battCol = persist.tile([128, 1], f32, tag="battCol")
        nc.gpsimd.partition_broadcast(wattB, wattRowB)
        nc.gpsimd.partition_broadcast(battCol, battRow)

        # localT = local^T (f32r), then projW = local @ W_apair (f32r)
        localT = persist.tile([128, H], f32r, tag="localT")
        for hb in range(2):
            tp = outpp.tile([128, H], f32, tag="outp")
            nc.tensor.transpose(tp[:, 0:128], localSb[:, ts(hb, 128)], identity)
            nc.scalar.copy(out=localT[:, ts(hb, 128)], in_=tp[:, 0:128])
        pp = outpp.tile([128, H], f32, tag="outp")
        nc.tensor.matmul(pp, lhsT=localT[:, 0:128], rhs=wap0, start=True, stop=False)
        nc.tensor.matmul(pp, lhsT=localT[:, 128:256], rhs=wap1, start=False, stop=True)
        projW = persist.tile([128, H], f32r, tag="projW")
        nc.scalar.copy(out=projW, in_=pp)
        projWr = projW

        # projFlat: bounce projW through DRAM, reload folded (one strided DMA)
        projDram = dramp.tile([N, H], f32r, tag="projDram")
        nc.sync.dma_start(out=projDram, in_=projW)
        projFlat = persist.tile([97, 32 * H], f32r, tag="projFlat")
        nc.sync.dma_start(
            out=projFlat[0:97:32, :],
            in_=projDram.rearrange("(a x) h -> a (x h)", a=4),
        )

        # binp[q][j, (i32, c32)]: c 0..15 = binary[., i, j, .], c16 = 1.0 (bias
        # lane).  Four separate quarter tiles so their loads can stagger into
        # the chunk loop without false dependencies.
        binp = []
        for q in range(4):
            bq = persist.tile([128, 32 * CPAD], f32, tag=f"binp{q}")
            nc.gpsimd.memset(bq, 0.0)
            nc.gpsimd.memset(
                bq.rearrange("p (i c) -> p i c", c=CPAD)[:, :, 16:17], 1.0
            )
            binp.append(bq)

        def load_binp(q):
            nc.sync.dma_start(
                out=binp[q].rearrange("p (i c) -> p i c", c=CPAD)[:, :, 0:BIN],
                in_=binary_d[ts(q, 32)].rearrange("i j c -> j i c"),
            )

        logits = persist.tile([128, N], f32, tag="logits")
        ttrS = persist.tile([128, H], bf16, tag="ttrS")
        binT = {}

        # ---------------- helpers ----------------
        def row_rhs(flat, i, width):
            q, r = divmod(i, 32)
            return flat[32 * q : 32 * q + 1, r * H : r * H + width]

        def row_lhsT(i):
            q = i // 32
            return onesT[32 * q : 32 * q + 1, :]

        def row_tp(i):
            return (32 * (i // 32), 0)

        projW2 = projWr.unsqueeze(1).broadcast_to([128, 2, H])

        def attn_step(i):
            g, il = divmod(i, IG)
            if il == 0:  # transpose this binary group: [j,(i4,c32)] -> [(i4,c32),j]
                tp = outpp.tile([128, H], f32, tag="outp")
                nc.tensor.transpose(
                    tp[:, 0:128], binp[g // 8][:, ts(g % 8, 128)], identity
                )
                bt = binTp.tile([128, 128], bf16, tag="binT")
                nc.scalar.copy(out=bt, in_=tp[:, 0:128])
                binT[g] = bt
            if i % 2 == 1:
                return
            pre = prep.tile([128, 2 * H], f32, tag="pre")
            nc.tensor.matmul(pre, lhsT=row_lhsT(i), rhs=row_rhs(projFlat, i, 2 * H),
                             start=True, stop=False, tile_position=row_tp(i))
            for m in range(2):
                ii = i + m
                gg, iil = divmod(ii, IG)
                nc.tensor.matmul(pre[:, ts(m, H)], lhsT=identR, rhs=projWr,
                                 start=False, stop=False)
                nc.tensor.matmul(
                    pre[:, ts(m, H)],
                    lhsT=binT[gg][32 * iil : 32 * iil + 17, :],
                    rhs=wx4[32 * iil : 32 * iil + 17, :],
                    start=False, stop=(m == 1), tile_position=(32 * iil, 0),
                )
            a2 = att2p.tile([128, 2 * H], bf16, tag="att2")
            nc.scalar.activation(out=a2, in_=pre, func=Relu)
            for m in range(2):
                nc.vector.affine_mul_reduce(
                    out=ttrS, accum_out=logits[:, i + m : i + m + 1],
                    in0=a2[:, ts(m, H)], in1=wattB, scale=1.0, bias=0.0,
                )

        # ---------------- output phase ----------------
        # binp quarter q feeds attention i-ticks [32q, 32q+32); issue its load
        # a few chunks ahead of first use.
        BINP_AT = {8: 1, 22: 2, 36: 3} if STAGGER_BINP else {}

        def out_phase(xSb, xR, src_flat, dram_flat, with_attn):
            x4 = xSb.unsqueeze(1).broadcast_to([128, CHUNK, H])
            x2 = (xR if xR is not None else xSb).unsqueeze(1).broadcast_to(
                [128, 2, H])
            variants = VAR_P1 if with_attn else VAR_P2
            attn_at = 0

            def attn_tick(limit):
                nonlocal attn_at
                if with_attn:
                    while attn_at < min(limit, N):
                        attn_step(attn_at)
                        attn_at += 1

            for nchunk in range(N // CHUNK):
                jj = CHUNK * nchunk
                v = variants[nchunk % len(variants)]
                po = genp.tile([128, CHUNK * H], f32, tag="gen")
                for b in range(CHUNK // 2):
                    if v == "E":
                        nc.tensor.matmul(
                            po[:, ts(b, 2 * H)], lhsT=identR, rhs=x2,
                            start=True, stop=False,
                        )
                    nc.tensor.matmul(
                        po[:, ts(b, 2 * H)], lhsT=row_lhsT(jj),
                        rhs=row_rhs(src_flat, jj + 2 * b, 2 * H),
                        start=(v != "E"), stop=True, tile_position=row_tp(jj),
                    )
                stage = stagep.tile([128, CHUNK * H], f32, tag="stage")
                st3 = stage.rearrange("p (j h) -> p j h", h=H)
                po3 = po.rearrange("p (j h) -> p j h", h=H)
                if v == "D":
                    nc.vector.tensor_add(out=st3, in0=x4, in1=po3)
                else:  # E: both terms already in PSUM
                    nc.scalar.copy(out=st3, in_=po3)
                nc.sync.dma_start(
                    out=dram_flat[:, jj * H : (jj + CHUNK) * H], in_=stage
                )
                if with_attn and nchunk in BINP_AT:
                    load_binp(BINP_AT[nchunk])
                # 128 attn i-ticks paced over the first ~56 of 64 chunks so
                # glob resolves before the phase-1 store tail.
                attn_tick(((nchunk + 1) * 16) // 7 - 4)
            attn_tick(N)

        # ---------------- phase 1: local_pair + attention ----------------
        load_binp(0)
        if not STAGGER_BINP:
            for q in (1, 2, 3):
                load_binp(q)
        xR1 = persist.tile([N, H], f32r, tag="xR1")
        nc.vector.tensor_copy(out=xR1, in_=localSb)
        out_phase(localSb, xR1, flatX, lp_flat, with_attn=True)

        # ---------------- scores -> glob ----------------
        scoreT = persist.tile([128, N], f32, tag="scoreT")
        globSb = persist.tile([128, H], f32, tag="globSb")
        nc.scalar.activation(out=scoreT, in_=logits, func=Sigmoid, bias=battCol)
        pg = outpp.tile([128, H], f32, tag="outp")
        nc.tensor.matmul(pg, lhsT=scoreT, rhs=localSb, start=True, stop=True)
        nc.vector.tensor_copy(out=globSb, in_=pg)
        globR = persist.tile([128, H], f32r, tag="globR")
        nc.scalar.copy(out=globR, in_=pg)
        globDram = dramp.tile([N, H], f32r, tag="globDram")
        nc.sync.dma_start(out=globDram, in_=globR)
        flatG = persist.tile([97, 32 * H], f32r, tag="flatG")
        nc.sync.dma_start(
            out=flatG[0:97:32, :],
            in_=globDram.rearrange("(a x) h -> a (x h)", a=4),
        )

        # ---------------- phase 2: global_pair ----------------
        out_phase(globSb, globR, flatG, gp_flat, with_attn=False)


def _build(reps=1):
    import concourse.bass as bass  # noqa: F401
    from concourse import bacc
    import concourse.mybir as mybir
    import concourse.tile as tile

    f32 = mybir.dt.float32
    nc = bacc.Bacc(
        "TRN2",
        target_bir_lowering=False,
        debug=False,
        enable_asserts=False,
        num_devices=NCORES,
    )
    io = (
        nc.dram_tensor("local", [N, H], f32, kind="ExternalInput").ap(),
        nc.dram_tensor("binary", [N, N, BIN], f32, kind="ExternalInput").ap(),
        nc.dram_tensor("w_apair", [H, H], f32, kind="ExternalInput").ap(),
        nc.dram_tensor("b_apair", [H], f32, kind="ExternalInput").ap(),
        nc.dram_tensor("w_binary", [BIN, H], f32, kind="ExternalInput").ap(),
        nc.dram_tensor("b_binary", [H], f32, kind="ExternalInput").ap(),
        nc.dram_tensor("w_att", [H, 1], f32, kind="ExternalInput").ap(),
        nc.dram_tensor("b_att", [1], f32, kind="ExternalInput").ap(),
        nc.dram_tensor("out_lp", [N, N, H], f32, kind="ExternalOutput").ap(),
        nc.dram_tensor("out_gp", [N, N, H], f32, kind="ExternalOutput").ap(),
    )
    with tile.TileContext(nc) as tc:
        _body(tc, io, reps=reps)
    nc.compile()
    return nc


def _get_nc():
    if "nc" not in _cache:
        _cache["nc"] = _build()
    return _cache["nc"]


def _run(inputs, trace=False):
    from concourse.bass_utils import run_bass_kernel_spmd

    nc = _get_nc()
    f = lambda x: np.ascontiguousarray(np.asarray(x), dtype=np.float32)
    shared = {
        "w_apair": f(inputs["W_apair"]),
        "b_apair": f(inputs["b_apair"]),
        "w_binary": f(inputs["W_binary"]),
        "b_binary": f(inputs["b_binary"]),
        "w_att": f(inputs["W_att"]),
        "b_att": f(inputs["b_att"]),
    }
    local = f(inputs["local_feats"])
    binary = f(inputs["binary_feats"])
    in_maps = [
        {"local": local[c], "binary": binary[c], **shared} for c in range(NCORES)
    ]
    res = run_bass_kernel_spmd(
        nc, in_maps, core_ids=list(range(NCORES)), trace=trace
    )
    lp = np.stack([r["out_lp"] for r in res.results])
    gp = np.stack([r["out_gp"] for r in res.results])
    return (lp, gp), res


def kernel(**inputs):
    out, _ = _run(inputs, trace=False)
    return out
